# revision 24
# baseline (speedup 1.0000x reference)
"""Trainium2 Bass kernel for nn_Encoder_6339371729763.

6-layer shared-weight transformer encoder, B=4, S=2048, D=512, F=2048.
All 8 attention heads are identical -> attention is a single head with
HD=64 and tile(a, H) @ Wo collapses to a @ sum_of_Wo_blocks.

Sharding: 8 cores = 4 batch elements x 2 sequence halves; each core owns
Sq=1024 query rows of one batch element and AllGathers k^T/v^T with its
pair core each layer.

Layout: the residual stream (out/att/y) is fp32 — quantizing it to bf16
costs ~7e-3 rel err over 6 layers; every matmul operand (x^T, att^T,
q/k/v, e, relu(h), all weights) is bf16, which costs only ~3e-3 total
and runs the PE at full rate with fast weight loads.

Schedule: one software pipeline per layer over the two query halves.
Attention is emitted as 16 ET/exp/aT chunk-pairs ordered [local h0,
local h1, remote h0, remote h1] so eight pairs (~9us) of local work
cover the k/v AllGather, with the wo/LN1/att^T chain of h0 interleaved
into the last remote pairs; FFN1 of one half overlaps the other half's
wo/LN1 chain; FFN2 passes complete token-tiles early so LN2 and the
next layer's x^T transposes and q/k/v projections fill the layer
boundary.  This keeps the PE queue dense (HAM clock gate stays at
2.4GHz) and hides the scalar-engine exp/relu chains under PE work.
Engine split: PE matmuls/transposes, ACT exp/relu/evacs, DVE
residual+LN chains and PSUM evacuations.
"""
import sys
import numpy as np

if "/opt/trn_rl_repo" not in sys.path:
    sys.path.insert(0, "/opt/trn_rl_repo")

import concourse.bass as bass
import concourse.tile as tile
from concourse import bacc, mybir
from concourse.bass_utils import run_bass_kernel_spmd
from concourse.masks import make_identity

F32 = mybir.dt.float32
BF16 = mybir.dt.bfloat16
I32 = mybir.dt.int32
AF = mybir.ActivationFunctionType
ALU = mybir.AluOpType

B, S, D, H, F, L = 4, 2048, 512, 8, 2048, 6
HD = D // H          # 64
EPS = 1e-5
N_CORES = 8
SQ = S // 2          # 1024 rows per core
NT = SQ // 128       # 8 row tiles per core
DC = D // 128        # 4
FC = F // 128        # 16

_cache = {}


def _pos_encoding():
    pos = np.arange(S, dtype=np.float32).reshape(-1, 1)
    freqs = (0.0001 ** (2 * (np.arange(D, dtype=np.float32) // 2) / D)).reshape(1, -1)
    pe = pos * freqs
    pe[::2] = np.cos(pe[::2])
    pe[1::2] = np.sin(pe[1::2])
    return pe  # [S, D]


def _build():
    nc = bacc.Bacc(
        "TRN2",
        target_bir_lowering=False,
        debug=False,
        enable_asserts=True,
        num_devices=N_CORES,
    )
    X = nc.dram_tensor("X", [SQ, D], BF16, kind="ExternalInput").ap()
    Wqkv = nc.dram_tensor("Wqkv", [DC, 128, 3 * HD], BF16, kind="ExternalInput").ap()
    Wop = nc.dram_tensor("Wop", [HD, D], BF16, kind="ExternalInput").ap()
    Wf1 = nc.dram_tensor("Wf1", [DC, 128, F], BF16, kind="ExternalInput").ap()
    Wf2 = nc.dram_tensor("Wf2", [FC, 128, D], BF16, kind="ExternalInput").ap()
    OUT = nc.dram_tensor("OUT", [SQ, D], F32, kind="ExternalOutput").ap()

    with tile.TileContext(nc) as tc:
        with (
            tc.tile_pool(name="wpool", bufs=1) as wp,
            tc.tile_pool(name="state", bufs=1) as st,
            tc.tile_pool(name="roll", bufs=2) as rl,
            tc.tile_pool(name="psA", bufs=2, space="PSUM") as psA,
            tc.tile_pool(name="psB", bufs=2, space="PSUM") as psB,
            tc.tile_pool(name="psW", bufs=2, space="PSUM") as psW,
            tc.tile_pool(name="dram", bufs=2, space="DRAM") as dram,
        ):
            # ---------------- inputs (X first: prologue needs it) ----------
            xbf = st.tile([128, NT, D], BF16)      # bf16 initial stream
            nc.sync.dma_start(xbf[:], X.rearrange("(t p) d -> p t d", p=128))

            wqkv_sb = wp.tile([128, DC, 3 * HD], BF16)
            for c in range(DC):
                nc.sync.dma_start(wqkv_sb[:, c, :], Wqkv[c])
            wop_sb = wp.tile([128, D], BF16)
            nc.vector.memset(wop_sb[:], 0.0)
            nc.sync.dma_start(wop_sb[0:HD, :], Wop[:])
            wf1_sb = wp.tile([128, DC, F], BF16)
            for c in range(DC):
                nc.sync.dma_start(wf1_sb[:, c, :], Wf1[c])
            wf2_sb = wp.tile([128, FC, D], BF16)
            for c in range(FC):
                nc.sync.dma_start(wf2_sb[:, c, :], Wf2[c])

            ident32 = wp.tile([128, 128], F32)
            make_identity(nc, ident32[:])
            ident_b = wp.tile([128, 128], BF16)
            nc.vector.tensor_copy(ident_b[:], ident32[:])
            # bf16 identity at partitions 64-127 (for base-64 v transposes)
            id64_32 = wp.tile([128, 64], F32)
            nc.vector.memset(id64_32[:], 0.0)
            nc.sync.dma_start(id64_32[64:128, :], ident32[0:64, 0:64])
            identr64 = wp.tile([128, 64], BF16)
            nc.vector.tensor_copy(identr64[:], id64_32[:])

            # v_aug: [keys 128, chunk 16, 128]; col HD all-ones (softmax
            # denom), cols HD+1.. stay zero (aT rows 65-127 = zeros)
            v_aug = wp.tile([128, 2 * NT, 128], BF16)
            nc.vector.memset(v_aug[:], 0.0)
            ones32 = wp.tile([128, 2 * NT], F32)
            nc.vector.memset(ones32[:], 1.0)
            nc.vector.tensor_copy(v_aug[:, :, HD], ones32[:])

            outf32 = wp.tile([128, NT, D], F32)   # final-layer staging

            # partner row offset in the flattened AllGather output
            pid = nc.partition_id(
                engines=[mybir.EngineType.Pool, mybir.EngineType.SP]
            )
            poff = (1 - (pid & 1)) * 128

            # ---------------- state tiles ----------------
            out_sb = st.tile([128, NT, D], F32)    # residual stream (fp32)
            att_sb = st.tile([128, NT, D], F32)
            y_sb = st.tile([128, NT, D], F32)
            xt_sb = st.tile([128, DC, SQ], BF16)   # x^T
            at_sb = st.tile([128, DC, SQ], BF16)   # att^T
            qt_sb = st.tile([128, SQ], BF16)       # q^T at rows 0:64 AND 64:128
            k2l_sb = st.tile([128, SQ], BF16)      # local kT copy at rows 64:128
            kr_sb = st.tile([128, SQ], BF16)       # partner kv (kT 0:64, vT 64:128)
            k2r_sb = st.tile([128, SQ], BF16)      # remote kT copy at rows 64:128
            aT_sb = st.tile([128, 2, 512], BF16)   # attn out^T per q-half
            rs_sb = st.tile([128, NT], F32)
            recip_sb = st.tile([128, NT], F32)
            sums = st.tile([128, NT], F32)         # per-tile sum(y)
            ssq = st.tile([128, NT], F32)          # per-tile sum(y^2)
            ysq_scr = st.tile([128, D], F32)       # y^2 scratch
            mean_t = st.tile([128, NT], F32)
            var_t = st.tile([128, NT], F32)
            nwt_t = st.tile([128, NT], F32)
            nwt_h = st.tile([128, NT], F32)
            rstd1 = st.tile([128, NT], F32)
            negm1 = st.tile([128, NT], F32)
            rstd2 = st.tile([128, NT], F32)
            negm2 = st.tile([128, NT], F32)

            def newton_rsqrt(v_ap, out_ap, t_ap, h_ap):
                """out = 1/sqrt(v), v > 0, on DVE."""
                nc.vector.tensor_scalar(
                    t_ap.bitcast(I32), v_ap.bitcast(I32), 1, None,
                    ALU.arith_shift_right,
                )
                nc.vector.tensor_scalar(
                    out_ap.bitcast(I32), t_ap.bitcast(I32), -1, 0x5F3759DF,
                    ALU.mult, op1=ALU.add,
                )
                for _ in range(2):
                    nc.vector.tensor_mul(h_ap, out_ap, out_ap)
                    nc.vector.tensor_mul(h_ap, h_ap, v_ap)
                    nc.vector.tensor_scalar(h_ap, h_ap, -0.5, 1.5, ALU.mult, op1=ALU.add)
                    nc.vector.tensor_mul(out_ap, out_ap, h_ap)

            def ln_stats(t):
                """DVE: sum(y^2) for tile t (sum(y) rides the residual op)."""
                nc.vector.scalar_tensor_tensor(
                    ysq_scr[:], y_sb[:, t, :], 1.0, y_sb[:, t, :],
                    op0=ALU.mult, op1=ALU.mult, accum_out=ssq[:, t:t + 1],
                )

            def ln_finish(t0, t1, rstd, negm):
                """DVE: rstd/negm for tiles t0:t1 from sums/ssq."""
                nc.vector.tensor_scalar(
                    mean_t[:, t0:t1], sums[:, t0:t1], 1.0 / D, None, ALU.mult
                )
                nc.vector.tensor_mul(
                    nwt_h[:, t0:t1], mean_t[:, t0:t1], mean_t[:, t0:t1]
                )
                # var + eps = ssq/D - mean^2 + eps
                nc.vector.scalar_tensor_tensor(
                    var_t[:, t0:t1], ssq[:, t0:t1], 1.0 / D, nwt_h[:, t0:t1],
                    op0=ALU.mult, op1=ALU.subtract,
                )
                nc.vector.tensor_scalar(
                    var_t[:, t0:t1], var_t[:, t0:t1], EPS, None, ALU.add
                )
                newton_rsqrt(
                    var_t[:, t0:t1], rstd[:, t0:t1], nwt_t[:, t0:t1], nwt_h[:, t0:t1]
                )
                nc.vector.tensor_tensor(
                    negm[:, t0:t1], mean_t[:, t0:t1], rstd[:, t0:t1], op=ALU.mult
                )
                nc.vector.tensor_scalar(
                    negm[:, t0:t1], negm[:, t0:t1], -1.0, None, ALU.mult
                )

            def ln_apply(t, dst, rstd, negm):
                nc.vector.tensor_scalar(
                    dst[:, t, :], y_sb[:, t, :], rstd[:, t:t + 1], negm[:, t:t + 1],
                    ALU.mult, op1=ALU.add,
                )

            def transpose_tile(src_tile, dst_tile, t, layer, nm, evac):
                """dst[:, :, t*128:(t+1)*128] = src[:, t, :]^T (PE + evac)."""
                dt_ = src_tile[:, t, :].dtype
                trp = psW.tile(
                    [128, D], dt_, tag="psW", name=f"{nm}_{layer}_{t}"
                )
                ident = ident_b if dt_ == BF16 else ident32
                for pt in range(DC):
                    nc.tensor.transpose(
                        trp[:, pt * 128:(pt + 1) * 128],
                        src_tile[:, t, pt * 128:(pt + 1) * 128],
                        ident[:],
                    )
                view = trp[:].rearrange("p (c n) -> p c n", c=DC)
                dst = dst_tile[:, :, t * 128:(t + 1) * 128]
                if evac == "act":
                    nc.scalar.activation(dst, view, AF.Copy)
                else:
                    nc.vector.tensor_copy(dst, view)

            def vtranspose(src, vbase, layer, nm, j0=0, j1=NT):
                """v_aug[:, vbase+j0:vbase+j1, 0:HD] = vT chunks of src."""
                vtp = psA.tile(
                    [128, (j1 - j0) * 64], BF16, tag="psA",
                    name=f"vtp_{nm}_{layer}",
                )
                for j in range(j0, j1):
                    nc.tensor.transpose(
                        vtp[:, (j - j0) * 64:(j - j0 + 1) * 64],
                        src[64:128, j * 128:(j + 1) * 128],
                        identr64[64:128, :],
                    )
                nc.scalar.activation(
                    v_aug[:, vbase + j0:vbase + j1, 0:HD], vtp[:], AF.Copy
                )

            def qkv_half(layer, kv_send, hx):
                """PE: k/v/q projections for token-half hx; ACT evac + dups."""
                n0, n1 = hx * 512, (hx + 1) * 512
                kvq = psA.tile(
                    [128, SQ], F32, tag="psA", name=f"kvq_{layer}_{hx}"
                )
                for c in range(DC):
                    nc.tensor.matmul(
                        kvq[:, 0:512],
                        wqkv_sb[:, c, 0:128],
                        xt_sb[:, c, n0:n1],
                        start=(c == 0), stop=(c == DC - 1),
                    )
                for c in range(DC):
                    nc.tensor.matmul(
                        kvq[0:64, 512:1024],
                        wqkv_sb[:, c, 128:192],
                        xt_sb[:, c, n0:n1],
                        start=(c == 0), stop=(c == DC - 1),
                    )
                nc.scalar.activation(kv_send[:, n0:n1], kvq[:, 0:512], AF.Copy)
                nc.scalar.activation(
                    qt_sb[0:64, n0:n1], kvq[0:64, 512:1024], AF.Copy
                )
                nc.sync.dma_start(qt_sb[64:128, n0:n1], qt_sb[0:64, n0:n1])
                nc.sync.dma_start(k2l_sb[64:128, n0:n1], kv_send[0:64, n0:n1])

            def rowsums(layer, qh):
                """PE: rowsums for q-half qh from aT_sb denom row; DVE recip."""
                rs_ps = psB.tile(
                    [128, 4, 2], BF16, tag="psB", name=f"rs_{layer}_{qh}"
                )
                for j in range(4):
                    nc.tensor.transpose(
                        rs_ps[:, j, 0:1],
                        aT_sb[HD:HD + 1, qh, j * 128:(j + 1) * 128],
                        identr64[64:65, 0:1],
                    )
                t0 = qh * 4
                nc.vector.tensor_copy(rs_sb[:, t0:t0 + 4], rs_ps[:, :, 0])
                nc.vector.reciprocal(recip_sb[:, t0:t0 + 4], rs_sb[:, t0:t0 + 4])

            def wo_tile(layer, qh, j):
                """PE: wo matmul for token tile j of half qh; DVE: residual."""
                t = qh * 4 + j
                wo_ps = psW.tile(
                    [128, D], F32, tag="psW", name=f"wo_{layer}_{t}"
                )
                nc.tensor.matmul(
                    wo_ps[:], aT_sb[:, qh, j * 128:(j + 1) * 128], wop_sb[:],
                    start=True, stop=True,
                )
                nc.vector.scalar_tensor_tensor(
                    y_sb[:, t, :], wo_ps[:], recip_sb[:, t:t + 1], out_sb[:, t, :],
                    op0=ALU.mult, op1=ALU.add, accum_out=sums[:, t:t + 1],
                )
                ln_stats(t)

            def attention(layer, kv_send, fills):
                """16 ET/exp/aT chunk-pairs: [loc h0, rem h0, loc h1, rem h1].

                Even chunks stream from src rows 0:64 (PE rows 0-63), odd
                chunks from the k2 dup at rows 64:128, so each pair's two ET
                matmuls run concurrently in disjoint row groups.  fills maps
                global pair index -> callables emitted after that pair; the
                h0 wo/LN1/attT chain and the first FFN1(h0) pairs ride in
                the h1 pair stream.
                """
                aT = [None, None]
                pend = []
                flushed = [0, 0]

                def flush():
                    e_sb, qh, iA, iB, first = pend.pop(0)
                    flushed[qh] += 1
                    nc.tensor.matmul(
                        aT[qh][:], v_aug[:, iA, :], e_sb[:, 0:512],
                        start=first, stop=False,
                    )
                    nc.tensor.matmul(
                        aT[qh][:], v_aug[:, iB, :], e_sb[:, 512:1024],
                        start=False, stop=(flushed[qh] == 8),
                    )

                seq = [(0, 0, 0), (0, 0, 1), (0, 0, 2), (0, 0, 3),
                       (0, 1, 0), (0, 1, 1), (1, 0, 0), (1, 0, 1),
                       (0, 1, 2), (0, 1, 3), (1, 0, 2), (1, 0, 3),
                       (1, 1, 0), (1, 1, 1), (1, 1, 2), (1, 1, 3)]
                for gi, (qh, rem, p) in enumerate(seq):
                    if gi == 4:
                        # partner token-half 0 k/v just landed (first AG)
                        vtranspose(kr_sb, NT, layer, "remA", 0, 4)
                    elif gi == 8:
                        vtranspose(kr_sb, NT, layer, "remB", 4, NT)
                    if rem == 0:
                        src, k2, vb = kv_send, k2l_sb, 0
                    else:
                        src, k2, vb = kr_sb, k2r_sb, NT
                    cA, cB = 2 * p, 2 * p + 1
                    n0, n1 = qh * 512, (qh + 1) * 512
                    if aT[qh] is None:
                        aT[qh] = psB.tile(
                            [128, 512], F32, tag="psB", name=f"aT_{layer}_{qh}"
                        )
                    et = psA.tile(
                        [128, SQ], F32, tag="psA", name=f"et_{layer}_{gi}"
                    )
                    nc.tensor.matmul(
                        et[:, 0:512],
                        src[0:64, cA * 128:(cA + 1) * 128],
                        qt_sb[0:64, n0:n1],
                        start=True, stop=True,
                    )
                    nc.tensor.matmul(
                        et[:, 512:1024],
                        k2[64:128, cB * 128:(cB + 1) * 128],
                        qt_sb[64:128, n0:n1],
                        start=True, stop=True,
                    )
                    e_sb = rl.tile(
                        [128, SQ], BF16, tag="e", bufs=4, name=f"e_{layer}_{gi}"
                    )
                    nc.scalar.activation(e_sb[:], et[:], AF.Exp, scale=0.125)
                    pend.append((e_sb, qh, vb + cA, vb + cB, rem == 0 and p == 0))
                    if len(pend) > 1:
                        flush()
                    for fn in fills.get(gi, ()):
                        fn(aT)
                while pend:
                    flush()
                return aT

            def ffn1_pair(layer, qh, fp, hrel):
                """PE: FFN1 matmuls for f-chunk pair fp; ACT: relu evac."""
                n0, n1 = qh * 512, (qh + 1) * 512
                h_ps = psA.tile(
                    [128, SQ], F32, tag="psA", name=f"h_{layer}_{qh}_{fp}"
                )
                for s in range(2):
                    fc = 2 * fp + s
                    for c in range(DC):
                        nc.tensor.matmul(
                            h_ps[:, s * 512:(s + 1) * 512],
                            wf1_sb[:, c, fc * 128:(fc + 1) * 128],
                            at_sb[:, c, n0:n1],
                            start=(c == 0), stop=(c == DC - 1),
                        )
                nc.scalar.activation(hrel[:, fp, :], h_ps[:], AF.Relu)

            def ffn2_pass(layer, qh, j, hrel):
                """PE: FFN2 accumulation for token tile j; DVE: residual+stats."""
                t = qh * 4 + j
                fq = psB.tile(
                    [128, D], F32, tag="psB", name=f"fq_{layer}_{t}"
                )
                for fc in range(FC):
                    nc.tensor.matmul(
                        fq[:],
                        hrel[:, fc // 2, (fc % 2) * 512 + j * 128:
                             (fc % 2) * 512 + (j + 1) * 128],
                        wf2_sb[:, fc, :],
                        start=(fc == 0), stop=(fc == FC - 1),
                    )
                nc.vector.scalar_tensor_tensor(
                    y_sb[:, t, :], fq[:], 1.0, att_sb[:, t, :],
                    op0=ALU.mult, op1=ALU.add, accum_out=sums[:, t:t + 1],
                )
                ln_stats(t)

            def ag_kick(layer, kv_send, half):
                """AllGather of one token-half of k/v with the pair core."""
                n0, n1 = half * 512, (half + 1) * 512
                cc_in = dram.tile(
                    [128, 512], BF16, tag=f"cin{half}",
                    name=f"cin{half}_{layer}",
                )
                nc.sync.dma_start(cc_in[:], kv_send[:, n0:n1])
                cc_out = dram.tile(
                    [256, 512], BF16, tag=f"cout{half}",
                    name=f"cout{half}_{layer}",
                )
                nc.gpsimd.collective_compute(
                    "AllGather",
                    ALU.bypass,
                    replica_groups=[[0, 1], [2, 3], [4, 5], [6, 7]],
                    ins=[cc_in.opt()],
                    outs=[cc_out.opt()],
                )
                nc.sync.dma_start(
                    kr_sb[:, n0:n1], cc_out[bass.ds(poff, 128), :]
                )
                nc.sync.dma_start(
                    k2r_sb[64:128, n0:n1], kr_sb[0:64, n0:n1]
                )

            # ---------------- prologue ----------------
            # warmup collective: absorbs CC-stack init + cross-core skew so
            # layer 0's real k/v AllGathers run at steady-state latency
            warm_sb = wp.tile([128, 2], BF16)
            nc.vector.memset(warm_sb[:], 0.0)
            warm_in = dram.tile([128, 2], BF16, tag="cwu", name="cwu_in")
            nc.sync.dma_start(warm_in[:], warm_sb[:])
            warm_out = dram.tile([256, 2], BF16, tag="cwo", name="cwu_out")
            nc.gpsimd.collective_compute(
                "AllGather",
                ALU.bypass,
                replica_groups=[[0, 1], [2, 3], [4, 5], [6, 7]],
                ins=[warm_in.opt()],
                outs=[warm_out.opt()],
            )

            # x^T for layer 0 from the bf16 input; fp32 residual via DVE
            for t in range(NT):
                transpose_tile(xbf, xt_sb, t, 99, "xt", "act")
            for t in range(NT):
                nc.vector.tensor_copy(out_sb[:, t, :], xbf[:, t, :])

            kv_next = rl.tile([128, SQ], BF16, tag="kvs", bufs=2, name="kvs_0")
            qkv_half(0, kv_next, 0)
            ag_kick(0, kv_next, 0)
            qkv_half(0, kv_next, 1)
            ag_kick(0, kv_next, 1)

            for layer in range(L):
                last = layer == L - 1
                kv_send = kv_next

                vtranspose(kv_send, 0, layer, "loc")

                hrel0 = rl.tile(
                    [128, NT, SQ], BF16, tag="hrel", bufs=2,
                    name=f"hrel_{layer}_0",
                )

                # ---- attention; h0's wo/LN1 DVE chain rides in the h1
                # pair stream (no PE ops that could head-of-line block) ----
                def fill_h0(step):
                    def go(aT):
                        if step == 0:
                            nc.vector.tensor_copy(aT_sb[:, 0, :], aT[0][:])
                            rowsums(layer, 0)
                        elif step == 1:
                            for j in range(4):
                                wo_tile(layer, 0, j)
                        else:
                            ln_finish(0, 4, rstd1, negm1)
                    return go

                fills = {10: [fill_h0(0)], 11: [fill_h0(1)], 12: [fill_h0(2)]}
                aT = attention(layer, kv_send, fills)
                nc.vector.tensor_copy(aT_sb[:, 1, :], aT[1][:])
                for t in range(4):
                    ln_apply(t, att_sb, rstd1, negm1)
                    transpose_tile(att_sb, at_sb, t, layer, "at", "dve")

                # ---------- FFN1(h0) + wo/LN1/attT(h1) ----------
                def fill_h1(step):
                    if step == 0:
                        rowsums(layer, 1)
                        for j in range(4):
                            wo_tile(layer, 1, j)
                    elif step == 1:
                        ln_finish(4, 8, rstd1, negm1)
                    elif step == 4:
                        for t in (4, 5):
                            ln_apply(t, att_sb, rstd1, negm1)
                            transpose_tile(att_sb, at_sb, t, layer, "at", "dve")
                    elif step == 5:
                        for t in (6, 7):
                            ln_apply(t, att_sb, rstd1, negm1)
                            transpose_tile(att_sb, at_sb, t, layer, "at", "dve")

                for fp in range(NT):
                    ffn1_pair(layer, 0, fp, hrel0)
                    fill_h1(fp)

                # ---------- FFN2(h0), LN2(h0) stats per tile ----------
                for j in range(4):
                    ffn2_pass(layer, 0, j, hrel0)
                ln_finish(0, 4, rstd2, negm2)

                # ---------- FFN1(h1) + LN2(h0) apply + x^T(h0);
                # next layer's h0 q/k/v + its AllGather kick mid-loop ------
                hrel1 = rl.tile(
                    [128, NT, SQ], BF16, tag="hrel", bufs=2,
                    name=f"hrel_{layer}_1",
                )
                for fp in range(NT):
                    ffn1_pair(layer, 1, fp, hrel1)
                    if fp < 4:
                        t = fp
                        ln_apply(t, outf32 if last else out_sb, rstd2, negm2)
                        if not last:
                            transpose_tile(out_sb, xt_sb, t, layer, "xt", "act")
                    elif fp == 4 and not last:
                        kv_next = rl.tile(
                            [128, SQ], BF16, tag="kvs", bufs=2,
                            name=f"kvs_{layer + 1}",
                        )
                        qkv_half(layer + 1, kv_next, 0)
                        ag_kick(layer + 1, kv_next, 0)
                if last:
                    nc.sync.dma_start(
                        OUT.rearrange("(t p) d -> p t d", p=128)[:, 0:4, :],
                        outf32[:, 0:4, :],
                    )

                # ---------- FFN2(h1) with per-pass LN2(h1)/x^T(h1) so the
                # next layer's h1 k/v and its AllGather kick early ----------
                for j in range(4):
                    ffn2_pass(layer, 1, j, hrel1)
                    t = 4 + j
                    ln_finish(t, t + 1, rstd2, negm2)
                    ln_apply(t, outf32 if last else out_sb, rstd2, negm2)
                    if not last:
                        transpose_tile(out_sb, xt_sb, t, layer, "xt", "act")
                if last:
                    nc.sync.dma_start(
                        OUT.rearrange("(t p) d -> p t d", p=128)[:, 4:8, :],
                        outf32[:, 4:8, :],
                    )
                else:
                    qkv_half(layer + 1, kv_next, 1)
                    ag_kick(layer + 1, kv_next, 1)

    nc.compile()
    return nc


def _prep_inputs(X, Wq, bq, Wk, bk, Wv, bv, Wo, bo, Wf1, bf1, Wf2, bf2,
                 ln1_g, ln1_b, ln2_g, ln2_b):
    import ml_dtypes
    bf = ml_dtypes.bfloat16
    f32 = np.float32
    for name, arr, want in [
        ("bq", bq, 0.0), ("bk", bk, 0.0), ("bv", bv, 0.0), ("bo", bo, 0.0),
        ("bf1", bf1, 0.0), ("bf2", bf2, 0.0),
        ("ln1_b", ln1_b, 0.0), ("ln2_b", ln2_b, 0.0),
        ("ln1_g", ln1_g, 1.0), ("ln2_g", ln2_g, 1.0),
    ]:
        assert np.allclose(np.asarray(arr), want, atol=0.0), (
            f"kernel specialized for trivial {name}"
        )
    X_pe = (np.asarray(X, f32) + _pos_encoding()[None]).astype(bf)  # [B, S, D]
    Wqkv = np.concatenate(
        [np.asarray(Wk, f32), np.asarray(Wv, f32), np.asarray(Wq, f32)], axis=1
    ).reshape(DC, 128, 3 * HD).astype(bf)
    Wop = (
        np.asarray(Wo, f32).reshape(H, HD, D).astype(np.float64).sum(0)
    ).astype(bf)
    Wf1r = np.asarray(Wf1, f32).reshape(DC, 128, F).astype(bf)
    Wf2r = np.asarray(Wf2, f32).reshape(FC, 128, D).astype(bf)
    in_maps = []
    for core in range(N_CORES):
        b, h = core // 2, core % 2
        in_maps.append({
            "X": np.ascontiguousarray(X_pe[b, h * SQ:(h + 1) * SQ]),
            "Wqkv": Wqkv, "Wop": Wop, "Wf1": Wf1r, "Wf2": Wf2r,
        })
    return in_maps


def _get_nc():
    if "nc" not in _cache:
        _cache["nc"] = _build()
    return _cache["nc"]


def kernel(**inputs) -> np.ndarray:
    nc = _get_nc()
    in_maps = _prep_inputs(**inputs)
    _cache["in_maps"] = in_maps
    res = run_bass_kernel_spmd(nc, in_maps, core_ids=list(range(N_CORES)))
    shards = [res.results[c]["OUT"] for c in range(N_CORES)]
    out = np.stack(shards).reshape(B, 2, SQ, D).reshape(B, S, D)
    return out


def profile_exec_time():
    """Re-run with NTFF tracing enabled; returns exec_time_ns (test.py use)."""
    import types
    import antenv
    import concourse.bass_utils as bu

    if "antenv.axon_hooks" not in sys.modules:
        mod = types.ModuleType("antenv.axon_hooks")
        _state = {"hook": None}
        mod.set_axon_ntff_profile_hook = lambda h: _state.__setitem__("hook", h)
        mod.get_axon_ntff_profile_hook = lambda: _state["hook"]
        sys.modules["antenv.axon_hooks"] = mod
        antenv.axon_hooks = mod
        from trn_agent_boot.trn_boot import _ntff_profile_via_ctypes
        mod.set_axon_ntff_profile_hook(
            _ntff_profile_via_ctypes("/opt/axon/libaxon_pjrt.so")
        )
        bu.upload_artifacts = lambda tmpdir: tmpdir
    nc = _get_nc()
    in_maps = _cache["in_maps"]
    res = run_bass_kernel_spmd(
        nc, in_maps, core_ids=list(range(N_CORES)), trace=True, trace_cores=[0]
    )
    _cache["last_trace"] = res.instructions_and_trace
    _cache["last_res"] = res
    return res.exec_time_ns


# revision 28
# speedup vs baseline: 1.1078x; 1.1078x over previous
"""Trainium2 Bass kernel for nn_Encoder_6339371729763.

6-layer shared-weight transformer encoder, B=4, S=2048, D=512, F=2048.
All 8 attention heads are identical -> attention is a single head with
HD=64 and tile(a, H) @ Wo collapses to a @ sum_of_Wo_blocks.

Sharding: 8 cores = 4 batch elements x 2 sequence halves; each core owns
Sq=1024 query rows of one batch element and AllGathers k^T/v^T with its
pair core each layer.

Layout: the residual stream (out/att/y) is fp32 — quantizing it to bf16
costs ~7e-3 rel err over 6 layers; every matmul operand (x^T, att^T,
q/k/v, e, relu(h), all weights) is bf16, which costs only ~3e-3 total
and runs the PE at full rate with fast weight loads.

Schedule: one software pipeline per layer over the two query halves.
Attention is emitted as 16 ET/exp/aT chunk-pairs ordered [local h0,
local h1, remote h0, remote h1] so eight pairs (~9us) of local work
cover the k/v AllGather, with the wo/LN1/att^T chain of h0 interleaved
into the last remote pairs; FFN1 of one half overlaps the other half's
wo/LN1 chain; FFN2 passes complete token-tiles early so LN2 and the
next layer's x^T transposes and q/k/v projections fill the layer
boundary.  This keeps the PE queue dense (HAM clock gate stays at
2.4GHz) and hides the scalar-engine exp/relu chains under PE work.
Engine split: PE matmuls/transposes, ACT exp/relu/evacs, DVE
residual+LN chains and PSUM evacuations.
"""
import sys
import numpy as np

if "/opt/trn_rl_repo" not in sys.path:
    sys.path.insert(0, "/opt/trn_rl_repo")

import concourse.bass as bass
import concourse.tile as tile
from concourse import bacc, mybir
from concourse.bass_utils import run_bass_kernel_spmd
from concourse.masks import make_identity

F32 = mybir.dt.float32
BF16 = mybir.dt.bfloat16
I32 = mybir.dt.int32
AF = mybir.ActivationFunctionType
ALU = mybir.AluOpType

B, S, D, H, F, L = 4, 2048, 512, 8, 2048, 6
HD = D // H          # 64
EPS = 1e-5
N_CORES = 8
SQ = S // 2          # 1024 rows per core
NT = SQ // 128       # 8 row tiles per core
DC = D // 128        # 4
FC = F // 128        # 16

_cache = {}


def _pos_encoding():
    pos = np.arange(S, dtype=np.float32).reshape(-1, 1)
    freqs = (0.0001 ** (2 * (np.arange(D, dtype=np.float32) // 2) / D)).reshape(1, -1)
    pe = pos * freqs
    pe[::2] = np.cos(pe[::2])
    pe[1::2] = np.sin(pe[1::2])
    return pe  # [S, D]


def _build():
    nc = bacc.Bacc(
        "TRN2",
        target_bir_lowering=False,
        debug=False,
        enable_asserts=True,
        num_devices=N_CORES,
    )
    X = nc.dram_tensor("X", [SQ, D], BF16, kind="ExternalInput").ap()
    Wqkv = nc.dram_tensor("Wqkv", [DC, 128, 3 * HD], BF16, kind="ExternalInput").ap()
    Wop = nc.dram_tensor("Wop", [HD, D], BF16, kind="ExternalInput").ap()
    Wf1 = nc.dram_tensor("Wf1", [DC, 128, F], BF16, kind="ExternalInput").ap()
    Wf2 = nc.dram_tensor("Wf2", [FC, 128, D], BF16, kind="ExternalInput").ap()
    OUT = nc.dram_tensor("OUT", [SQ, D], F32, kind="ExternalOutput").ap()

    with tile.TileContext(nc) as tc:
        with (
            tc.tile_pool(name="wpool", bufs=1) as wp,
            tc.tile_pool(name="state", bufs=1) as st,
            tc.tile_pool(name="roll", bufs=2) as rl,
            tc.tile_pool(name="psA", bufs=2, space="PSUM") as psA,
            tc.tile_pool(name="psB", bufs=2, space="PSUM") as psB,
            tc.tile_pool(name="psW", bufs=2, space="PSUM") as psW,
            tc.tile_pool(name="dram", bufs=2, space="DRAM") as dram,
        ):
            # ---------------- inputs (X first: prologue needs it) ----------
            xbf = st.tile([128, NT, D], BF16)      # bf16 initial stream
            nc.sync.dma_start(xbf[:], X.rearrange("(t p) d -> p t d", p=128))

            wqkv_sb = wp.tile([128, DC, 3 * HD], BF16)
            for c in range(DC):
                nc.sync.dma_start(wqkv_sb[:, c, :], Wqkv[c])
            wop_sb = wp.tile([128, D], BF16)
            nc.vector.memset(wop_sb[:], 0.0)
            nc.sync.dma_start(wop_sb[0:HD, :], Wop[:])
            wf1_sb = wp.tile([128, DC, F], BF16)
            for c in range(DC):
                nc.sync.dma_start(wf1_sb[:, c, :], Wf1[c])
            wf2_sb = wp.tile([128, FC, D], BF16)
            for c in range(FC):
                nc.sync.dma_start(wf2_sb[:, c, :], Wf2[c])

            ident32 = wp.tile([128, 128], F32)
            make_identity(nc, ident32[:])
            ident_b = wp.tile([128, 128], BF16)
            nc.vector.tensor_copy(ident_b[:], ident32[:])
            # bf16 identity at partitions 64-127 (for base-64 v transposes)
            id64_32 = wp.tile([128, 64], F32)
            nc.vector.memset(id64_32[:], 0.0)
            nc.sync.dma_start(id64_32[64:128, :], ident32[0:64, 0:64])
            identr64 = wp.tile([128, 64], BF16)
            nc.vector.tensor_copy(identr64[:], id64_32[:])

            # v_aug: [keys 128, chunk 16, 128]; col HD all-ones (softmax
            # denom), cols HD+1.. stay zero (aT rows 65-127 = zeros)
            v_aug = wp.tile([128, 2 * NT, 128], BF16)
            nc.vector.memset(v_aug[:], 0.0)
            ones32 = wp.tile([128, 2 * NT], F32)
            nc.vector.memset(ones32[:], 1.0)
            nc.vector.tensor_copy(v_aug[:, :, HD], ones32[:])

            outf32 = wp.tile([128, NT, D], F32)   # final-layer staging

            # partner row offset in the flattened AllGather output
            pid = nc.partition_id(
                engines=[mybir.EngineType.Pool, mybir.EngineType.SP]
            )
            poff = (1 - (pid & 1)) * 128

            # ---------------- state tiles ----------------
            out_sb = st.tile([128, NT, D], F32)    # residual stream (fp32)
            att_sb = st.tile([128, NT, D], F32)
            y_sb = st.tile([128, NT, D], F32)
            xt_sb = st.tile([128, DC, SQ], BF16)   # x^T
            at_sb = st.tile([128, DC, SQ], BF16)   # att^T
            qt_sb = st.tile([128, SQ], BF16)       # q^T at rows 0:64 AND 64:128
            k2l_sb = st.tile([128, SQ], BF16)      # local kT copy at rows 64:128
            kr_sb = st.tile([128, SQ], BF16)       # partner kv (kT 0:64, vT 64:128)
            k2r_sb = st.tile([128, SQ], BF16)      # remote kT copy at rows 64:128
            aT_sb = st.tile([128, 2, 512], BF16)   # attn out^T per q-half
            rs_sb = st.tile([128, NT], F32)
            recip_sb = st.tile([128, NT], F32)
            sums = st.tile([128, NT], F32)         # per-tile sum(y)
            ssq = st.tile([128, NT], F32)          # per-tile sum(y^2)
            ysq_scr = st.tile([128, D], F32)       # y^2 scratch
            mean_t = st.tile([128, NT], F32)
            var_t = st.tile([128, NT], F32)
            nwt_t = st.tile([128, NT], F32)
            nwt_h = st.tile([128, NT], F32)
            rstd1 = st.tile([128, NT], F32)
            negm1 = st.tile([128, NT], F32)
            rstd2 = st.tile([128, NT], F32)
            negm2 = st.tile([128, NT], F32)

            def newton_rsqrt(v_ap, out_ap, t_ap, h_ap):
                """out = 1/sqrt(v), v > 0, on DVE."""
                nc.vector.tensor_scalar(
                    t_ap.bitcast(I32), v_ap.bitcast(I32), 1, None,
                    ALU.arith_shift_right,
                )
                nc.vector.tensor_scalar(
                    out_ap.bitcast(I32), t_ap.bitcast(I32), -1, 0x5F3759DF,
                    ALU.mult, op1=ALU.add,
                )
                for _ in range(2):
                    nc.vector.tensor_mul(h_ap, out_ap, out_ap)
                    nc.vector.tensor_mul(h_ap, h_ap, v_ap)
                    nc.vector.tensor_scalar(h_ap, h_ap, -0.5, 1.5, ALU.mult, op1=ALU.add)
                    nc.vector.tensor_mul(out_ap, out_ap, h_ap)

            def ln_stats(t):
                """DVE: sum(y^2) for tile t (sum(y) rides the residual op)."""
                nc.vector.scalar_tensor_tensor(
                    ysq_scr[:], y_sb[:, t, :], 1.0, y_sb[:, t, :],
                    op0=ALU.mult, op1=ALU.mult, accum_out=ssq[:, t:t + 1],
                )

            def ln_finish(t0, t1, rstd, negm):
                """DVE: rstd/negm for tiles t0:t1 from sums/ssq."""
                nc.vector.tensor_scalar(
                    mean_t[:, t0:t1], sums[:, t0:t1], 1.0 / D, None, ALU.mult
                )
                nc.vector.tensor_mul(
                    nwt_h[:, t0:t1], mean_t[:, t0:t1], mean_t[:, t0:t1]
                )
                # var + eps = ssq/D - mean^2 + eps
                nc.vector.scalar_tensor_tensor(
                    var_t[:, t0:t1], ssq[:, t0:t1], 1.0 / D, nwt_h[:, t0:t1],
                    op0=ALU.mult, op1=ALU.subtract,
                )
                nc.vector.tensor_scalar(
                    var_t[:, t0:t1], var_t[:, t0:t1], EPS, None, ALU.add
                )
                newton_rsqrt(
                    var_t[:, t0:t1], rstd[:, t0:t1], nwt_t[:, t0:t1], nwt_h[:, t0:t1]
                )
                nc.vector.tensor_tensor(
                    negm[:, t0:t1], mean_t[:, t0:t1], rstd[:, t0:t1], op=ALU.mult
                )
                nc.vector.tensor_scalar(
                    negm[:, t0:t1], negm[:, t0:t1], -1.0, None, ALU.mult
                )

            def ln_apply(t, dst, rstd, negm):
                nc.vector.tensor_scalar(
                    dst[:, t, :], y_sb[:, t, :], rstd[:, t:t + 1], negm[:, t:t + 1],
                    ALU.mult, op1=ALU.add,
                )

            def transpose_tile(src_tile, dst_tile, t, layer, nm, evac):
                """dst[:, :, t*128:(t+1)*128] = src[:, t, :]^T (PE + evac)."""
                dt_ = src_tile[:, t, :].dtype
                trp = psW.tile(
                    [128, D], dt_, tag="psW", name=f"{nm}_{layer}_{t}"
                )
                ident = ident_b if dt_ == BF16 else ident32
                for pt in range(DC):
                    nc.tensor.transpose(
                        trp[:, pt * 128:(pt + 1) * 128],
                        src_tile[:, t, pt * 128:(pt + 1) * 128],
                        ident[:],
                    )
                view = trp[:].rearrange("p (c n) -> p c n", c=DC)
                dst = dst_tile[:, :, t * 128:(t + 1) * 128]
                if evac == "act":
                    nc.scalar.activation(dst, view, AF.Copy)
                else:
                    nc.vector.tensor_copy(dst, view)

            def vtranspose(src, vbase, layer, nm, j0=0, j1=NT):
                """v_aug[:, vbase+j0:vbase+j1, 0:HD] = vT chunks of src."""
                vtp = psA.tile(
                    [128, (j1 - j0) * 64], BF16, tag="psA",
                    name=f"vtp_{nm}_{layer}",
                )
                for j in range(j0, j1):
                    nc.tensor.transpose(
                        vtp[:, (j - j0) * 64:(j - j0 + 1) * 64],
                        src[64:128, j * 128:(j + 1) * 128],
                        identr64[64:128, :],
                    )
                nc.scalar.activation(
                    v_aug[:, vbase + j0:vbase + j1, 0:HD], vtp[:], AF.Copy
                )

            def qkv_half(layer, kv_send, hx):
                """PE: k/v/q projections for token-half hx; ACT evac + dups."""
                n0, n1 = hx * 512, (hx + 1) * 512
                kvq = psA.tile(
                    [128, SQ], F32, tag="psA", name=f"kvq_{layer}_{hx}"
                )
                for c in range(DC):
                    nc.tensor.matmul(
                        kvq[:, 0:512],
                        wqkv_sb[:, c, 0:128],
                        xt_sb[:, c, n0:n1],
                        start=(c == 0), stop=(c == DC - 1),
                    )
                for c in range(DC):
                    nc.tensor.matmul(
                        kvq[0:64, 512:1024],
                        wqkv_sb[:, c, 128:192],
                        xt_sb[:, c, n0:n1],
                        start=(c == 0), stop=(c == DC - 1),
                    )
                nc.scalar.activation(kv_send[:, n0:n1], kvq[:, 0:512], AF.Copy)
                nc.scalar.activation(
                    qt_sb[0:64, n0:n1], kvq[0:64, 512:1024], AF.Copy
                )
                nc.sync.dma_start(qt_sb[64:128, n0:n1], qt_sb[0:64, n0:n1])
                nc.sync.dma_start(k2l_sb[64:128, n0:n1], kv_send[0:64, n0:n1])

            def rowsums(layer, qh):
                """PE: rowsums for q-half qh from aT_sb denom row; DVE recip."""
                rs_ps = psB.tile(
                    [128, 4, 2], BF16, tag="psB", name=f"rs_{layer}_{qh}"
                )
                for j in range(4):
                    nc.tensor.transpose(
                        rs_ps[:, j, 0:1],
                        aT_sb[HD:HD + 1, qh, j * 128:(j + 1) * 128],
                        identr64[64:65, 0:1],
                    )
                t0 = qh * 4
                nc.vector.tensor_copy(rs_sb[:, t0:t0 + 4], rs_ps[:, :, 0])
                nc.vector.reciprocal(recip_sb[:, t0:t0 + 4], rs_sb[:, t0:t0 + 4])

            def wo_tile(layer, qh, j):
                """PE: wo matmul for token tile j of half qh; DVE: residual."""
                t = qh * 4 + j
                wo_ps = psW.tile(
                    [128, D], F32, tag="psW", name=f"wo_{layer}_{t}"
                )
                nc.tensor.matmul(
                    wo_ps[:], aT_sb[:, qh, j * 128:(j + 1) * 128], wop_sb[:],
                    start=True, stop=True,
                )
                nc.vector.scalar_tensor_tensor(
                    y_sb[:, t, :], wo_ps[:], recip_sb[:, t:t + 1], out_sb[:, t, :],
                    op0=ALU.mult, op1=ALU.add, accum_out=sums[:, t:t + 1],
                )

            def center_half(qh):
                """DVE: att = y - mean(y) per token.  LN1's rstd cancels:
                relu/matmul/LN2 commute with positive per-row scaling, so
                only the mean-subtract of LN1 is needed."""
                t0 = qh * 4
                nc.vector.tensor_scalar(
                    mean_t[:, t0:t0 + 4], sums[:, t0:t0 + 4], 1.0 / D, None,
                    ALU.mult,
                )
                for t in range(t0, t0 + 4):
                    nc.vector.tensor_scalar(
                        att_sb[:, t, :], y_sb[:, t, :], mean_t[:, t:t + 1],
                        None, ALU.subtract,
                    )

            def attention(layer, kv_send, fills):
                """16 ET/exp/aT chunk-pairs: [loc h0, rem h0, loc h1, rem h1].

                Even chunks stream from src rows 0:64 (PE rows 0-63), odd
                chunks from the k2 dup at rows 64:128, so each pair's two ET
                matmuls run concurrently in disjoint row groups.  fills maps
                global pair index -> callables emitted after that pair; the
                h0 wo/LN1/attT chain and the first FFN1(h0) pairs ride in
                the h1 pair stream.
                """
                aT = [None, None]
                pend = []
                flushed = [0, 0]

                def flush():
                    e_sb, qh, iA, iB, first = pend.pop(0)
                    flushed[qh] += 1
                    nc.tensor.matmul(
                        aT[qh][:], v_aug[:, iA, :], e_sb[:, 0:512],
                        start=first, stop=False,
                    )
                    nc.tensor.matmul(
                        aT[qh][:], v_aug[:, iB, :], e_sb[:, 512:1024],
                        start=False, stop=(flushed[qh] == 8),
                    )

                seq = [(0, 0, 0), (0, 0, 1), (0, 0, 2), (0, 0, 3),
                       (0, 1, 0), (0, 1, 1), (1, 0, 0), (1, 0, 1),
                       (1, 0, 2), (1, 0, 3), (0, 1, 2), (0, 1, 3),
                       (1, 1, 0), (1, 1, 1), (1, 1, 2), (1, 1, 3)]
                for gi, (qh, rem, p) in enumerate(seq):
                    if gi == 4:
                        # partner token-half 0 k/v just landed (first AG)
                        vtranspose(kr_sb, NT, layer, "remA", 0, 4)
                    elif gi == 10:
                        vtranspose(kr_sb, NT, layer, "remB", 4, NT)
                    if rem == 0:
                        src, k2, vb = kv_send, k2l_sb, 0
                    else:
                        src, k2, vb = kr_sb, k2r_sb, NT
                    cA, cB = 2 * p, 2 * p + 1
                    n0, n1 = qh * 512, (qh + 1) * 512
                    if aT[qh] is None:
                        aT[qh] = psB.tile(
                            [128, 512], F32, tag="psB", name=f"aT_{layer}_{qh}"
                        )
                    et = psA.tile(
                        [128, SQ], F32, tag="psA", name=f"et_{layer}_{gi}"
                    )
                    nc.tensor.matmul(
                        et[:, 0:512],
                        src[0:64, cA * 128:(cA + 1) * 128],
                        qt_sb[0:64, n0:n1],
                        start=True, stop=True,
                    )
                    nc.tensor.matmul(
                        et[:, 512:1024],
                        k2[64:128, cB * 128:(cB + 1) * 128],
                        qt_sb[64:128, n0:n1],
                        start=True, stop=True,
                    )
                    e_sb = rl.tile(
                        [128, SQ], BF16, tag="e", bufs=4, name=f"e_{layer}_{gi}"
                    )
                    nc.scalar.activation(e_sb[:], et[:], AF.Exp, scale=0.125)
                    pend.append((e_sb, qh, vb + cA, vb + cB, rem == 0 and p == 0))
                    if len(pend) > 1:
                        flush()
                    for fn in fills.get(gi, ()):
                        fn(aT)
                while pend:
                    flush()
                return aT

            def ffn1_pair(layer, qh, fp, hrel):
                """PE: FFN1 matmuls for f-chunk pair fp; ACT: relu evac."""
                n0, n1 = qh * 512, (qh + 1) * 512
                h_ps = psA.tile(
                    [128, SQ], F32, tag="psA", name=f"h_{layer}_{qh}_{fp}"
                )
                for s in range(2):
                    fc = 2 * fp + s
                    for c in range(DC):
                        nc.tensor.matmul(
                            h_ps[:, s * 512:(s + 1) * 512],
                            wf1_sb[:, c, fc * 128:(fc + 1) * 128],
                            at_sb[:, c, n0:n1],
                            start=(c == 0), stop=(c == DC - 1),
                        )
                nc.scalar.activation(hrel[:, fp, :], h_ps[:], AF.Relu)

            def ffn2_pass(layer, qh, j, hrel):
                """PE: FFN2 accumulation for token tile j; DVE: residual+stats."""
                t = qh * 4 + j
                fq = psB.tile(
                    [128, D], F32, tag="psB", name=f"fq_{layer}_{t}"
                )
                for fc in range(FC):
                    nc.tensor.matmul(
                        fq[:],
                        hrel[:, fc // 2, (fc % 2) * 512 + j * 128:
                             (fc % 2) * 512 + (j + 1) * 128],
                        wf2_sb[:, fc, :],
                        start=(fc == 0), stop=(fc == FC - 1),
                    )
                nc.vector.scalar_tensor_tensor(
                    y_sb[:, t, :], fq[:], 1.0, att_sb[:, t, :],
                    op0=ALU.mult, op1=ALU.add, accum_out=sums[:, t:t + 1],
                )
                ln_stats(t)

            def ag_kick(layer, kv_send, half):
                """AllGather of one token-half of k/v with the pair core."""
                n0, n1 = half * 512, (half + 1) * 512
                cc_in = dram.tile(
                    [128, 512], BF16, tag=f"cin{half}",
                    name=f"cin{half}_{layer}",
                )
                nc.sync.dma_start(cc_in[:], kv_send[:, n0:n1])
                cc_out = dram.tile(
                    [256, 512], BF16, tag=f"cout{half}",
                    name=f"cout{half}_{layer}",
                )
                nc.gpsimd.collective_compute(
                    "AllGather",
                    ALU.bypass,
                    replica_groups=[[0, 1], [2, 3], [4, 5], [6, 7]],
                    ins=[cc_in.opt()],
                    outs=[cc_out.opt()],
                )
                nc.sync.dma_start(
                    kr_sb[:, n0:n1], cc_out[bass.ds(poff, 128), :]
                )
                nc.sync.dma_start(
                    k2r_sb[64:128, n0:n1], cc_out[bass.ds(poff, 64), :]
                )

            # ---------------- prologue ----------------
            # warmup collective: absorbs CC-stack init + cross-core skew so
            # layer 0's real k/v AllGathers run at steady-state latency
            warm_sb = wp.tile([128, 2], BF16)
            nc.vector.memset(warm_sb[:], 0.0)
            warm_in = dram.tile([128, 2], BF16, tag="cwu", name="cwu_in")
            nc.sync.dma_start(warm_in[:], warm_sb[:])
            warm_out = dram.tile([256, 2], BF16, tag="cwo", name="cwu_out")
            nc.gpsimd.collective_compute(
                "AllGather",
                ALU.bypass,
                replica_groups=[[0, 1], [2, 3], [4, 5], [6, 7]],
                ins=[warm_in.opt()],
                outs=[warm_out.opt()],
            )

            # x^T for layer 0 from the bf16 input; fp32 residual via DVE
            for t in range(NT):
                transpose_tile(xbf, xt_sb, t, 99, "xt", "act")
            for t in range(NT):
                nc.vector.tensor_copy(out_sb[:, t, :], xbf[:, t, :])

            kv_next = rl.tile([128, SQ], BF16, tag="kvs", bufs=2, name="kvs_0")
            qkv_half(0, kv_next, 0)
            ag_kick(0, kv_next, 0)
            qkv_half(0, kv_next, 1)
            ag_kick(0, kv_next, 1)

            for layer in range(L):
                last = layer == L - 1
                kv_send = kv_next

                vtranspose(kv_send, 0, layer, "loc")

                hrel0 = rl.tile(
                    [128, NT, SQ], BF16, tag="hrel", bufs=2,
                    name=f"hrel_{layer}_0",
                )

                # ---- attention; h0's wo/center DVE chain rides in the h1
                # pair stream (no PE ops that could head-of-line block) ----
                def fill_h0(step):
                    def go(aT):
                        if step == 0:
                            nc.vector.tensor_copy(aT_sb[:, 0, :], aT[0][:])
                            rowsums(layer, 0)
                        elif step == 1:
                            for j in range(4):
                                wo_tile(layer, 0, j)
                        else:
                            center_half(0)
                    return go

                fills = {12: [fill_h0(0)], 13: [fill_h0(1)], 14: [fill_h0(2)]}
                aT = attention(layer, kv_send, fills)
                nc.vector.tensor_copy(aT_sb[:, 1, :], aT[1][:])
                for t in range(4):
                    transpose_tile(att_sb, at_sb, t, layer, "at", "act")

                # ---------- FFN1(h0) + wo/center/attT(h1) ----------
                def fill_h1(step):
                    if step == 0:
                        rowsums(layer, 1)
                        for j in range(4):
                            wo_tile(layer, 1, j)
                    elif step == 1:
                        center_half(1)
                    elif step == 2:
                        for t in (4, 5):
                            transpose_tile(att_sb, at_sb, t, layer, "at", "dve")
                    elif step == 3:
                        for t in (6, 7):
                            transpose_tile(att_sb, at_sb, t, layer, "at", "dve")

                for fp in range(NT):
                    ffn1_pair(layer, 0, fp, hrel0)
                    fill_h1(fp)

                # ---------- FFN2(h0), LN2(h0) stats per tile ----------
                for j in range(4):
                    ffn2_pass(layer, 0, j, hrel0)
                ln_finish(0, 4, rstd2, negm2)

                # ---------- FFN1(h1) + LN2(h0) apply + x^T(h0);
                # next layer's h0 q/k/v + its AllGather kick mid-loop ------
                hrel1 = rl.tile(
                    [128, NT, SQ], BF16, tag="hrel", bufs=2,
                    name=f"hrel_{layer}_1",
                )
                for fp in range(NT):
                    ffn1_pair(layer, 1, fp, hrel1)
                    if fp < 4:
                        t = fp
                        ln_apply(t, outf32 if last else out_sb, rstd2, negm2)
                        if not last:
                            transpose_tile(out_sb, xt_sb, t, layer, "xt", "act")
                    elif fp == 4 and not last:
                        kv_next = rl.tile(
                            [128, SQ], BF16, tag="kvs", bufs=2,
                            name=f"kvs_{layer + 1}",
                        )
                        qkv_half(layer + 1, kv_next, 0)
                        ag_kick(layer + 1, kv_next, 0)
                if last:
                    nc.sync.dma_start(
                        OUT.rearrange("(t p) d -> p t d", p=128)[:, 0:4, :],
                        outf32[:, 0:4, :],
                    )

                # ---------- FFN2(h1) with per-pass LN2(h1)/x^T(h1) so the
                # next layer's h1 k/v and its AllGather kick early ----------
                for j in range(4):
                    ffn2_pass(layer, 1, j, hrel1)
                    t = 4 + j
                    ln_finish(t, t + 1, rstd2, negm2)
                    ln_apply(t, outf32 if last else out_sb, rstd2, negm2)
                    if not last:
                        transpose_tile(out_sb, xt_sb, t, layer, "xt", "act")
                if last:
                    nc.sync.dma_start(
                        OUT.rearrange("(t p) d -> p t d", p=128)[:, 4:8, :],
                        outf32[:, 4:8, :],
                    )
                else:
                    qkv_half(layer + 1, kv_next, 1)
                    ag_kick(layer + 1, kv_next, 1)

    nc.compile()
    return nc


def _prep_inputs(X, Wq, bq, Wk, bk, Wv, bv, Wo, bo, Wf1, bf1, Wf2, bf2,
                 ln1_g, ln1_b, ln2_g, ln2_b):
    import ml_dtypes
    bf = ml_dtypes.bfloat16
    f32 = np.float32
    for name, arr, want in [
        ("bq", bq, 0.0), ("bk", bk, 0.0), ("bv", bv, 0.0), ("bo", bo, 0.0),
        ("bf1", bf1, 0.0), ("bf2", bf2, 0.0),
        ("ln1_b", ln1_b, 0.0), ("ln2_b", ln2_b, 0.0),
        ("ln1_g", ln1_g, 1.0), ("ln2_g", ln2_g, 1.0),
    ]:
        assert np.allclose(np.asarray(arr), want, atol=0.0), (
            f"kernel specialized for trivial {name}"
        )
    X_pe = (np.asarray(X, f32) + _pos_encoding()[None]).astype(bf)  # [B, S, D]
    Wqkv = np.concatenate(
        [np.asarray(Wk, f32), np.asarray(Wv, f32), np.asarray(Wq, f32)], axis=1
    ).reshape(DC, 128, 3 * HD).astype(bf)
    Wop = (
        np.asarray(Wo, f32).reshape(H, HD, D).astype(np.float64).sum(0)
    ).astype(bf)
    Wf1r = np.asarray(Wf1, f32).reshape(DC, 128, F).astype(bf)
    Wf2r = np.asarray(Wf2, f32).reshape(FC, 128, D).astype(bf)
    in_maps = []
    for core in range(N_CORES):
        b, h = core // 2, core % 2
        in_maps.append({
            "X": np.ascontiguousarray(X_pe[b, h * SQ:(h + 1) * SQ]),
            "Wqkv": Wqkv, "Wop": Wop, "Wf1": Wf1r, "Wf2": Wf2r,
        })
    return in_maps


def _get_nc():
    if "nc" not in _cache:
        _cache["nc"] = _build()
    return _cache["nc"]


def kernel(**inputs) -> np.ndarray:
    nc = _get_nc()
    in_maps = _prep_inputs(**inputs)
    _cache["in_maps"] = in_maps
    res = run_bass_kernel_spmd(nc, in_maps, core_ids=list(range(N_CORES)))
    shards = [res.results[c]["OUT"] for c in range(N_CORES)]
    out = np.stack(shards).reshape(B, 2, SQ, D).reshape(B, S, D)
    return out


def profile_exec_time():
    """Re-run with NTFF tracing enabled; returns exec_time_ns (test.py use)."""
    import types
    import antenv
    import concourse.bass_utils as bu

    if "antenv.axon_hooks" not in sys.modules:
        mod = types.ModuleType("antenv.axon_hooks")
        _state = {"hook": None}
        mod.set_axon_ntff_profile_hook = lambda h: _state.__setitem__("hook", h)
        mod.get_axon_ntff_profile_hook = lambda: _state["hook"]
        sys.modules["antenv.axon_hooks"] = mod
        antenv.axon_hooks = mod
        from trn_agent_boot.trn_boot import _ntff_profile_via_ctypes
        mod.set_axon_ntff_profile_hook(
            _ntff_profile_via_ctypes("/opt/axon/libaxon_pjrt.so")
        )
        bu.upload_artifacts = lambda tmpdir: tmpdir
    nc = _get_nc()
    in_maps = _cache["in_maps"]
    res = run_bass_kernel_spmd(
        nc, in_maps, core_ids=list(range(N_CORES)), trace=True, trace_cores=[0]
    )
    _cache["last_trace"] = res.instructions_and_trace
    _cache["last_res"] = res
    return res.exec_time_ns


# revision 32
# speedup vs baseline: 1.1118x; 1.0036x over previous
"""Trainium2 Bass kernel for nn_Encoder_6339371729763.

6-layer shared-weight transformer encoder, B=4, S=2048, D=512, F=2048.
All 8 attention heads are identical -> attention is a single head with
HD=64 and tile(a, H) @ Wo collapses to a @ sum_of_Wo_blocks.

Sharding: 8 cores = 4 batch elements x 2 sequence halves; each core owns
Sq=1024 query rows of one batch element and AllGathers k^T/v^T with its
pair core each layer.

Layout: the residual stream (out/att/y) is fp32 — quantizing it to bf16
costs ~7e-3 rel err over 6 layers; every matmul operand (x^T, att^T,
q/k/v, e, relu(h), all weights) is bf16, which costs only ~3e-3 total
and runs the PE at full rate with fast weight loads.

Schedule: one software pipeline per layer over the two query halves.
Attention is emitted as 16 ET/exp/aT chunk-pairs ordered [local h0,
local h1, remote h0, remote h1] so eight pairs (~9us) of local work
cover the k/v AllGather, with the wo/LN1/att^T chain of h0 interleaved
into the last remote pairs; FFN1 of one half overlaps the other half's
wo/LN1 chain; FFN2 passes complete token-tiles early so LN2 and the
next layer's x^T transposes and q/k/v projections fill the layer
boundary.  This keeps the PE queue dense (HAM clock gate stays at
2.4GHz) and hides the scalar-engine exp/relu chains under PE work.
Engine split: PE matmuls/transposes, ACT exp/relu/evacs, DVE
residual+LN chains and PSUM evacuations.
"""
import sys
import numpy as np

if "/opt/trn_rl_repo" not in sys.path:
    sys.path.insert(0, "/opt/trn_rl_repo")

import concourse.bass as bass
import concourse.tile as tile
from concourse import bacc, mybir
from concourse.bass_utils import run_bass_kernel_spmd
from concourse.masks import make_identity

F32 = mybir.dt.float32
BF16 = mybir.dt.bfloat16
I32 = mybir.dt.int32
AF = mybir.ActivationFunctionType
ALU = mybir.AluOpType

B, S, D, H, F, L = 4, 2048, 512, 8, 2048, 6
HD = D // H          # 64
EPS = 1e-5
N_CORES = 8
SQ = S // 2          # 1024 rows per core
NT = SQ // 128       # 8 row tiles per core
DC = D // 128        # 4
FC = F // 128        # 16

_cache = {}


def _pos_encoding():
    pos = np.arange(S, dtype=np.float32).reshape(-1, 1)
    freqs = (0.0001 ** (2 * (np.arange(D, dtype=np.float32) // 2) / D)).reshape(1, -1)
    pe = pos * freqs
    pe[::2] = np.cos(pe[::2])
    pe[1::2] = np.sin(pe[1::2])
    return pe  # [S, D]


def _build():
    nc = bacc.Bacc(
        "TRN2",
        target_bir_lowering=False,
        debug=False,
        enable_asserts=True,
        num_devices=N_CORES,
    )
    X = nc.dram_tensor("X", [SQ, D], BF16, kind="ExternalInput").ap()
    Wqkv = nc.dram_tensor("Wqkv", [DC, 128, 3 * HD], BF16, kind="ExternalInput").ap()
    Wop = nc.dram_tensor("Wop", [HD, D], BF16, kind="ExternalInput").ap()
    Wf1 = nc.dram_tensor("Wf1", [DC, 128, F], BF16, kind="ExternalInput").ap()
    Wf2 = nc.dram_tensor("Wf2", [FC, 128, D], BF16, kind="ExternalInput").ap()
    OUT = nc.dram_tensor("OUT", [SQ, D], F32, kind="ExternalOutput").ap()

    with tile.TileContext(nc) as tc:
        with (
            tc.tile_pool(name="wpool", bufs=1) as wp,
            tc.tile_pool(name="state", bufs=1) as st,
            tc.tile_pool(name="roll", bufs=2) as rl,
            tc.tile_pool(name="psA", bufs=2, space="PSUM") as psA,
            tc.tile_pool(name="psB", bufs=2, space="PSUM") as psB,
            tc.tile_pool(name="psW", bufs=2, space="PSUM") as psW,
            tc.tile_pool(name="dram", bufs=2, space="DRAM") as dram,
        ):
            # ---------------- inputs (X first: prologue needs it) ----------
            xbf = st.tile([128, NT, D], BF16)      # bf16 initial stream
            nc.sync.dma_start(xbf[:], X.rearrange("(t p) d -> p t d", p=128))

            wqkv_sb = wp.tile([128, DC, 3 * HD], BF16)
            for c in range(DC):
                nc.sync.dma_start(wqkv_sb[:, c, :], Wqkv[c])
            wop_sb = wp.tile([128, D], BF16)
            nc.vector.memset(wop_sb[:], 0.0)
            nc.sync.dma_start(wop_sb[0:HD, :], Wop[:])
            wf1_sb = wp.tile([128, DC, F], BF16)
            for c in range(DC):
                nc.sync.dma_start(wf1_sb[:, c, :], Wf1[c])
            wf2_sb = wp.tile([128, FC, D], BF16)
            for c in range(FC):
                nc.sync.dma_start(wf2_sb[:, c, :], Wf2[c])

            ident32 = wp.tile([128, 128], F32)
            make_identity(nc, ident32[:])
            ident_b = wp.tile([128, 128], BF16)
            nc.vector.tensor_copy(ident_b[:], ident32[:])
            # bf16 identity at partitions 64-127 (for base-64 v transposes)
            id64_32 = wp.tile([128, 64], F32)
            nc.vector.memset(id64_32[:], 0.0)
            nc.sync.dma_start(id64_32[64:128, :], ident32[0:64, 0:64])
            identr64 = wp.tile([128, 64], BF16)
            nc.vector.tensor_copy(identr64[:], id64_32[:])

            # v_aug: [keys 128, chunk 16, 128]; col HD all-ones (softmax
            # denom), cols HD+1.. stay zero (aT rows 65-127 = zeros)
            v_aug = wp.tile([128, 2 * NT, 128], BF16)
            nc.vector.memset(v_aug[:], 0.0)
            ones32 = wp.tile([128, 2 * NT], F32)
            nc.vector.memset(ones32[:], 1.0)
            nc.vector.tensor_copy(v_aug[:, :, HD], ones32[:])

            outf32 = wp.tile([128, NT, D], F32)   # final-layer staging

            # partner row offset in the flattened AllGather output
            pid = nc.partition_id(
                engines=[mybir.EngineType.Pool, mybir.EngineType.SP]
            )
            poff = (1 - (pid & 1)) * 128

            # ---------------- state tiles ----------------
            out_sb = st.tile([128, NT, D], F32)    # residual stream (fp32)
            att_sb = st.tile([128, NT, D], F32)
            y_sb = st.tile([128, NT, D], F32)
            xt_sb = st.tile([128, DC, SQ], BF16)   # x^T
            at_sb = st.tile([128, DC, SQ], BF16)   # att^T
            qt_sb = st.tile([128, SQ], BF16)       # q^T at rows 0:64 AND 64:128
            k2l_sb = st.tile([128, SQ], BF16)      # local kT copy at rows 64:128
            kr_sb = st.tile([128, SQ], BF16)       # partner kv (kT 0:64, vT 64:128)
            k2r_sb = st.tile([128, SQ], BF16)      # remote kT copy at rows 64:128
            aT_sb = st.tile([128, 2, 512], BF16)   # attn out^T per q-half
            rs_sb = st.tile([128, NT], F32)
            recip_sb = st.tile([128, NT], F32)
            sums = st.tile([128, NT], F32)         # per-tile sum(y)
            ssq = st.tile([128, NT], F32)          # per-tile sum(y^2)
            ysq_scr = st.tile([128, D], F32)       # y^2 scratch
            mean_t = st.tile([128, NT], F32)
            var_t = st.tile([128, NT], F32)
            nwt_t = st.tile([128, NT], F32)
            nwt_h = st.tile([128, NT], F32)
            rstd1 = st.tile([128, NT], F32)
            negm1 = st.tile([128, NT], F32)
            rstd2 = st.tile([128, NT], F32)
            negm2 = st.tile([128, NT], F32)

            def newton_rsqrt(v_ap, out_ap, t_ap, h_ap):
                """out = 1/sqrt(v), v > 0, on DVE."""
                nc.vector.tensor_scalar(
                    t_ap.bitcast(I32), v_ap.bitcast(I32), 1, None,
                    ALU.arith_shift_right,
                )
                nc.vector.tensor_scalar(
                    out_ap.bitcast(I32), t_ap.bitcast(I32), -1, 0x5F3759DF,
                    ALU.mult, op1=ALU.add,
                )
                for _ in range(2):
                    nc.vector.tensor_mul(h_ap, out_ap, out_ap)
                    nc.vector.tensor_mul(h_ap, h_ap, v_ap)
                    nc.vector.tensor_scalar(h_ap, h_ap, -0.5, 1.5, ALU.mult, op1=ALU.add)
                    nc.vector.tensor_mul(out_ap, out_ap, h_ap)

            def ln_stats(t):
                """DVE: sum(y^2) for tile t (sum(y) rides the residual op)."""
                nc.vector.scalar_tensor_tensor(
                    ysq_scr[:], y_sb[:, t, :], 1.0, y_sb[:, t, :],
                    op0=ALU.mult, op1=ALU.mult, accum_out=ssq[:, t:t + 1],
                )

            def ln_finish(t0, t1, rstd, negm):
                """DVE: rstd/negm for tiles t0:t1 from sums/ssq."""
                nc.vector.tensor_scalar(
                    mean_t[:, t0:t1], sums[:, t0:t1], 1.0 / D, None, ALU.mult
                )
                nc.vector.tensor_mul(
                    nwt_h[:, t0:t1], mean_t[:, t0:t1], mean_t[:, t0:t1]
                )
                # var + eps = ssq/D - mean^2 + eps
                nc.vector.scalar_tensor_tensor(
                    var_t[:, t0:t1], ssq[:, t0:t1], 1.0 / D, nwt_h[:, t0:t1],
                    op0=ALU.mult, op1=ALU.subtract,
                )
                nc.vector.tensor_scalar(
                    var_t[:, t0:t1], var_t[:, t0:t1], EPS, None, ALU.add
                )
                newton_rsqrt(
                    var_t[:, t0:t1], rstd[:, t0:t1], nwt_t[:, t0:t1], nwt_h[:, t0:t1]
                )
                nc.vector.tensor_tensor(
                    negm[:, t0:t1], mean_t[:, t0:t1], rstd[:, t0:t1], op=ALU.mult
                )
                nc.vector.tensor_scalar(
                    negm[:, t0:t1], negm[:, t0:t1], -1.0, None, ALU.mult
                )

            def ln_apply(t, dst, rstd, negm):
                nc.vector.tensor_scalar(
                    dst[:, t, :], y_sb[:, t, :], rstd[:, t:t + 1], negm[:, t:t + 1],
                    ALU.mult, op1=ALU.add,
                )

            def transpose_tile(src_tile, dst_tile, t, layer, nm, evac):
                """dst[:, :, t*128:(t+1)*128] = src[:, t, :]^T (PE + evac)."""
                dt_ = src_tile[:, t, :].dtype
                trp = psW.tile(
                    [128, D], dt_, tag="psW", name=f"{nm}_{layer}_{t}"
                )
                ident = ident_b if dt_ == BF16 else ident32
                for pt in range(DC):
                    nc.tensor.transpose(
                        trp[:, pt * 128:(pt + 1) * 128],
                        src_tile[:, t, pt * 128:(pt + 1) * 128],
                        ident[:],
                    )
                view = trp[:].rearrange("p (c n) -> p c n", c=DC)
                dst = dst_tile[:, :, t * 128:(t + 1) * 128]
                if evac == "act":
                    nc.scalar.activation(dst, view, AF.Copy)
                else:
                    nc.vector.tensor_copy(dst, view)

            def vtranspose(src, vbase, layer, nm, j0=0, j1=NT):
                """v_aug[:, vbase+j0:vbase+j1, 0:HD] = vT chunks of src."""
                vtp = psA.tile(
                    [128, (j1 - j0) * 64], BF16, tag="psA",
                    name=f"vtp_{nm}_{layer}",
                )
                for j in range(j0, j1):
                    nc.tensor.transpose(
                        vtp[:, (j - j0) * 64:(j - j0 + 1) * 64],
                        src[64:128, j * 128:(j + 1) * 128],
                        identr64[64:128, :],
                    )
                nc.vector.tensor_copy(
                    v_aug[:, vbase + j0:vbase + j1, 0:HD], vtp[:]
                )

            def qkv_half(layer, kv_send, hx):
                """PE: k/v/q projections for token-half hx; ACT evac + dups."""
                n0, n1 = hx * 512, (hx + 1) * 512
                kvq = psA.tile(
                    [128, SQ], F32, tag="psA", name=f"kvq_{layer}_{hx}"
                )
                for c in range(DC):
                    nc.tensor.matmul(
                        kvq[:, 0:512],
                        wqkv_sb[:, c, 0:128],
                        xt_sb[:, c, n0:n1],
                        start=(c == 0), stop=(c == DC - 1),
                    )
                for c in range(DC):
                    nc.tensor.matmul(
                        kvq[0:64, 512:1024],
                        wqkv_sb[:, c, 128:192],
                        xt_sb[:, c, n0:n1],
                        start=(c == 0), stop=(c == DC - 1),
                    )
                nc.scalar.activation(kv_send[:, n0:n1], kvq[:, 0:512], AF.Copy)
                nc.scalar.activation(
                    qt_sb[0:64, n0:n1], kvq[0:64, 512:1024], AF.Copy
                )
                nc.sync.dma_start(qt_sb[64:128, n0:n1], qt_sb[0:64, n0:n1])
                nc.sync.dma_start(k2l_sb[64:128, n0:n1], kv_send[0:64, n0:n1])

            def rowsums(layer, qh):
                """PE: rowsums for q-half qh from aT_sb denom row; DVE recip."""
                rs_ps = psB.tile(
                    [128, 4, 2], BF16, tag="psB", name=f"rs_{layer}_{qh}"
                )
                for j in range(4):
                    nc.tensor.transpose(
                        rs_ps[:, j, 0:1],
                        aT_sb[HD:HD + 1, qh, j * 128:(j + 1) * 128],
                        identr64[64:65, 0:1],
                    )
                t0 = qh * 4
                nc.vector.tensor_copy(rs_sb[:, t0:t0 + 4], rs_ps[:, :, 0])
                nc.vector.reciprocal(recip_sb[:, t0:t0 + 4], rs_sb[:, t0:t0 + 4])

            def wo_tile(layer, qh, j):
                """PE: wo matmul for token tile j of half qh; DVE: residual."""
                t = qh * 4 + j
                wo_ps = psW.tile(
                    [128, D], F32, tag="psW", name=f"wo_{layer}_{t}"
                )
                nc.tensor.matmul(
                    wo_ps[:], aT_sb[:, qh, j * 128:(j + 1) * 128], wop_sb[:],
                    start=True, stop=True,
                )
                nc.vector.scalar_tensor_tensor(
                    y_sb[:, t, :], wo_ps[:], recip_sb[:, t:t + 1], out_sb[:, t, :],
                    op0=ALU.mult, op1=ALU.add, accum_out=sums[:, t:t + 1],
                )

            def center_half(qh):
                """DVE: att = y - mean(y) per token.  LN1's rstd cancels:
                relu/matmul/LN2 commute with positive per-row scaling, so
                only the mean-subtract of LN1 is needed."""
                t0 = qh * 4
                nc.vector.tensor_scalar(
                    mean_t[:, t0:t0 + 4], sums[:, t0:t0 + 4], 1.0 / D, None,
                    ALU.mult,
                )
                for t in range(t0, t0 + 4):
                    nc.vector.tensor_scalar(
                        att_sb[:, t, :], y_sb[:, t, :], mean_t[:, t:t + 1],
                        None, ALU.subtract,
                    )

            def attention(layer, kv_send, fills):
                """16 ET/exp/aT chunk-pairs: [loc h0, rem h0, loc h1, rem h1].

                Even chunks stream from src rows 0:64 (PE rows 0-63), odd
                chunks from the k2 dup at rows 64:128, so each pair's two ET
                matmuls run concurrently in disjoint row groups.  fills maps
                global pair index -> callables emitted after that pair; the
                h0 wo/LN1/attT chain and the first FFN1(h0) pairs ride in
                the h1 pair stream.
                """
                aT = [None, None]
                pend = []
                flushed = [0, 0]

                def flush():
                    e_sb, qh, iA, iB, first = pend.pop(0)
                    flushed[qh] += 1
                    nc.tensor.matmul(
                        aT[qh][:], v_aug[:, iA, :], e_sb[:, 0:512],
                        start=first, stop=False,
                    )
                    nc.tensor.matmul(
                        aT[qh][:], v_aug[:, iB, :], e_sb[:, 512:1024],
                        start=False, stop=(flushed[qh] == 8),
                    )

                seq = [(0, 0, 0), (0, 0, 1), (0, 0, 2), (0, 0, 3),
                       (0, 1, 0), (0, 1, 1), (1, 0, 0), (1, 0, 1),
                       (1, 0, 2), (1, 0, 3), (0, 1, 2), (0, 1, 3),
                       (1, 1, 0), (1, 1, 1), (1, 1, 2), (1, 1, 3)]
                for gi, (qh, rem, p) in enumerate(seq):
                    if gi == 4:
                        # partner token-half 0 k/v just landed (first AG)
                        vtranspose(kr_sb, NT, layer, "remA", 0, 4)
                    elif gi == 10:
                        vtranspose(kr_sb, NT, layer, "remB", 4, NT)
                    if rem == 0:
                        src, k2, vb = kv_send, k2l_sb, 0
                    else:
                        src, k2, vb = kr_sb, k2r_sb, NT
                    cA, cB = 2 * p, 2 * p + 1
                    n0, n1 = qh * 512, (qh + 1) * 512
                    if aT[qh] is None:
                        aT[qh] = psB.tile(
                            [128, 512], F32, tag="psB", name=f"aT_{layer}_{qh}"
                        )
                    et = psA.tile(
                        [128, SQ], F32, tag="psA", name=f"et_{layer}_{gi}"
                    )
                    nc.tensor.matmul(
                        et[:, 0:512],
                        src[0:64, cA * 128:(cA + 1) * 128],
                        qt_sb[0:64, n0:n1],
                        start=True, stop=True,
                    )
                    nc.tensor.matmul(
                        et[:, 512:1024],
                        k2[64:128, cB * 128:(cB + 1) * 128],
                        qt_sb[64:128, n0:n1],
                        start=True, stop=True,
                    )
                    e_sb = rl.tile(
                        [128, SQ], BF16, tag="e", bufs=4, name=f"e_{layer}_{gi}"
                    )
                    nc.scalar.activation(e_sb[:], et[:], AF.Exp, scale=0.125)
                    pend.append((e_sb, qh, vb + cA, vb + cB, rem == 0 and p == 0))
                    if len(pend) > 1:
                        flush()
                    for fn in fills.get(gi, ()):
                        fn(aT)
                while pend:
                    flush()
                return aT

            def ffn1_pair(layer, qh, fp, hrel):
                """PE: FFN1 matmuls for f-chunk pair fp; ACT: relu evac."""
                n0, n1 = qh * 512, (qh + 1) * 512
                h_ps = psA.tile(
                    [128, SQ], F32, tag="psA", name=f"h_{layer}_{qh}_{fp}"
                )
                for s in range(2):
                    fc = 2 * fp + s
                    for c in range(DC):
                        nc.tensor.matmul(
                            h_ps[:, s * 512:(s + 1) * 512],
                            wf1_sb[:, c, fc * 128:(fc + 1) * 128],
                            at_sb[:, c, n0:n1],
                            start=(c == 0), stop=(c == DC - 1),
                        )
                nc.scalar.activation(hrel[:, fp, :], h_ps[:], AF.Relu)

            def ffn2_pass(layer, qh, j, hrel):
                """PE: FFN2 accumulation for token tile j; DVE: residual+stats."""
                t = qh * 4 + j
                fq = psB.tile(
                    [128, D], F32, tag="psB", name=f"fq_{layer}_{t}"
                )
                for fc in range(FC):
                    nc.tensor.matmul(
                        fq[:],
                        hrel[:, fc // 2, (fc % 2) * 512 + j * 128:
                             (fc % 2) * 512 + (j + 1) * 128],
                        wf2_sb[:, fc, :],
                        start=(fc == 0), stop=(fc == FC - 1),
                    )
                nc.vector.scalar_tensor_tensor(
                    y_sb[:, t, :], fq[:], 1.0, att_sb[:, t, :],
                    op0=ALU.mult, op1=ALU.add, accum_out=sums[:, t:t + 1],
                )
                ln_stats(t)

            def ag_kick(layer, kv_send, half):
                """AllGather of one token-half of k/v with the pair core."""
                n0, n1 = half * 512, (half + 1) * 512
                cc_in = dram.tile(
                    [128, 512], BF16, tag=f"cin{half}",
                    name=f"cin{half}_{layer}",
                )
                nc.sync.dma_start(cc_in[:], kv_send[:, n0:n1])
                cc_out = dram.tile(
                    [256, 512], BF16, tag=f"cout{half}",
                    name=f"cout{half}_{layer}",
                )
                nc.gpsimd.collective_compute(
                    "AllGather",
                    ALU.bypass,
                    replica_groups=[[0, 1], [2, 3], [4, 5], [6, 7]],
                    ins=[cc_in.opt()],
                    outs=[cc_out.opt()],
                )
                nc.sync.dma_start(
                    kr_sb[:, n0:n1], cc_out[bass.ds(poff, 128), :]
                )
                nc.sync.dma_start(
                    k2r_sb[64:128, n0:n1], cc_out[bass.ds(poff, 64), :]
                )

            # ---------------- prologue ----------------
            # warmup collective: absorbs CC-stack init + cross-core skew so
            # layer 0's real k/v AllGathers run at steady-state latency
            warm_sb = wp.tile([128, 2], BF16)
            nc.vector.memset(warm_sb[:], 0.0)
            warm_in = dram.tile([128, 2], BF16, tag="cwu", name="cwu_in")
            nc.sync.dma_start(warm_in[:], warm_sb[:])
            warm_out = dram.tile([256, 2], BF16, tag="cwo", name="cwu_out")
            nc.gpsimd.collective_compute(
                "AllGather",
                ALU.bypass,
                replica_groups=[[0, 1], [2, 3], [4, 5], [6, 7]],
                ins=[warm_in.opt()],
                outs=[warm_out.opt()],
            )

            # x^T for layer 0 from the bf16 input; fp32 residual via DVE
            for t in range(NT):
                transpose_tile(xbf, xt_sb, t, 99, "xt", "act")
            for t in range(NT):
                nc.vector.tensor_copy(out_sb[:, t, :], xbf[:, t, :])

            kv_next = rl.tile([128, SQ], BF16, tag="kvs", bufs=2, name="kvs_0")
            qkv_half(0, kv_next, 0)
            ag_kick(0, kv_next, 0)
            qkv_half(0, kv_next, 1)
            ag_kick(0, kv_next, 1)

            for layer in range(L):
                last = layer == L - 1
                kv_send = kv_next

                vtranspose(kv_send, 0, layer, "loc")

                hrel0 = rl.tile(
                    [128, NT, SQ], BF16, tag="hrel", bufs=2,
                    name=f"hrel_{layer}_0",
                )

                # ---- attention; h0's wo/center DVE chain rides in the h1
                # pair stream (no PE ops that could head-of-line block) ----
                def fill_h0(step):
                    def go(aT):
                        if step == 0:
                            nc.vector.tensor_copy(aT_sb[:, 0, :], aT[0][:])
                            rowsums(layer, 0)
                        elif step == 1:
                            for j in range(4):
                                wo_tile(layer, 0, j)
                        else:
                            center_half(0)
                    return go

                fills = {12: [fill_h0(0)], 13: [fill_h0(1)], 14: [fill_h0(2)]}
                aT = attention(layer, kv_send, fills)
                nc.vector.tensor_copy(aT_sb[:, 1, :], aT[1][:])
                for t in range(4):
                    transpose_tile(att_sb, at_sb, t, layer, "at",
                                   "act" if t < 2 else "dve")

                # ---------- FFN1(h0) + wo/center/attT(h1) ----------
                def fill_h1(step):
                    if step == 0:
                        rowsums(layer, 1)
                        for j in range(4):
                            wo_tile(layer, 1, j)
                    elif step == 1:
                        center_half(1)
                    elif step == 2:
                        for t in (4, 5):
                            transpose_tile(att_sb, at_sb, t, layer, "at", "dve")
                    elif step == 3:
                        for t in (6, 7):
                            transpose_tile(att_sb, at_sb, t, layer, "at", "dve")

                for fp in range(NT):
                    ffn1_pair(layer, 0, fp, hrel0)
                    fill_h1(fp)

                # ---------- FFN2(h0), LN2(h0) stats per tile ----------
                for j in range(4):
                    ffn2_pass(layer, 0, j, hrel0)
                ln_finish(0, 4, rstd2, negm2)

                # ---------- FFN1(h1) + LN2(h0) apply + x^T(h0);
                # next layer's h0 q/k/v + its AllGather kick mid-loop ------
                hrel1 = rl.tile(
                    [128, NT, SQ], BF16, tag="hrel", bufs=2,
                    name=f"hrel_{layer}_1",
                )
                for fp in range(NT):
                    ffn1_pair(layer, 1, fp, hrel1)
                    if 1 <= fp <= 4:
                        # one-pair lag so the x^T transpose never head-of-line
                        # blocks the next FFN1 pair while LN2 applies drain
                        t = fp - 1
                        ln_apply(t, outf32 if last else out_sb, rstd2, negm2)
                        if not last:
                            transpose_tile(out_sb, xt_sb, t, layer, "xt", "act")
                    elif fp == 5 and not last:
                        kv_next = rl.tile(
                            [128, SQ], BF16, tag="kvs", bufs=2,
                            name=f"kvs_{layer + 1}",
                        )
                        qkv_half(layer + 1, kv_next, 0)
                        ag_kick(layer + 1, kv_next, 0)
                if last:
                    nc.sync.dma_start(
                        OUT.rearrange("(t p) d -> p t d", p=128)[:, 0:4, :],
                        outf32[:, 0:4, :],
                    )

                # ---------- FFN2(h1) with per-pass LN2(h1)/x^T(h1) so the
                # next layer's h1 k/v and its AllGather kick early ----------
                for j in range(4):
                    ffn2_pass(layer, 1, j, hrel1)
                    t = 4 + j
                    ln_finish(t, t + 1, rstd2, negm2)
                    ln_apply(t, outf32 if last else out_sb, rstd2, negm2)
                    if not last and j >= 1:
                        transpose_tile(out_sb, xt_sb, t - 1, layer, "xt", "act")
                if last:
                    nc.sync.dma_start(
                        OUT.rearrange("(t p) d -> p t d", p=128)[:, 4:8, :],
                        outf32[:, 4:8, :],
                    )
                else:
                    transpose_tile(out_sb, xt_sb, 7, layer, "xt", "act")
                    qkv_half(layer + 1, kv_next, 1)
                    ag_kick(layer + 1, kv_next, 1)

    nc.compile()
    return nc


def _prep_inputs(X, Wq, bq, Wk, bk, Wv, bv, Wo, bo, Wf1, bf1, Wf2, bf2,
                 ln1_g, ln1_b, ln2_g, ln2_b):
    import ml_dtypes
    bf = ml_dtypes.bfloat16
    f32 = np.float32
    for name, arr, want in [
        ("bq", bq, 0.0), ("bk", bk, 0.0), ("bv", bv, 0.0), ("bo", bo, 0.0),
        ("bf1", bf1, 0.0), ("bf2", bf2, 0.0),
        ("ln1_b", ln1_b, 0.0), ("ln2_b", ln2_b, 0.0),
        ("ln1_g", ln1_g, 1.0), ("ln2_g", ln2_g, 1.0),
    ]:
        assert np.allclose(np.asarray(arr), want, atol=0.0), (
            f"kernel specialized for trivial {name}"
        )
    X_pe = (np.asarray(X, f32) + _pos_encoding()[None]).astype(bf)  # [B, S, D]
    Wqkv = np.concatenate(
        [np.asarray(Wk, f32), np.asarray(Wv, f32), np.asarray(Wq, f32)], axis=1
    ).reshape(DC, 128, 3 * HD).astype(bf)
    Wop = (
        np.asarray(Wo, f32).reshape(H, HD, D).astype(np.float64).sum(0)
    ).astype(bf)
    Wf1r = np.asarray(Wf1, f32).reshape(DC, 128, F).astype(bf)
    Wf2r = np.asarray(Wf2, f32).reshape(FC, 128, D).astype(bf)
    in_maps = []
    for core in range(N_CORES):
        b, h = core // 2, core % 2
        in_maps.append({
            "X": np.ascontiguousarray(X_pe[b, h * SQ:(h + 1) * SQ]),
            "Wqkv": Wqkv, "Wop": Wop, "Wf1": Wf1r, "Wf2": Wf2r,
        })
    return in_maps


def _get_nc():
    if "nc" not in _cache:
        _cache["nc"] = _build()
    return _cache["nc"]


def kernel(**inputs) -> np.ndarray:
    nc = _get_nc()
    in_maps = _prep_inputs(**inputs)
    _cache["in_maps"] = in_maps
    res = run_bass_kernel_spmd(nc, in_maps, core_ids=list(range(N_CORES)))
    shards = [res.results[c]["OUT"] for c in range(N_CORES)]
    out = np.stack(shards).reshape(B, 2, SQ, D).reshape(B, S, D)
    return out


def profile_exec_time():
    """Re-run with NTFF tracing enabled; returns exec_time_ns (test.py use)."""
    import types
    import antenv
    import concourse.bass_utils as bu

    if "antenv.axon_hooks" not in sys.modules:
        mod = types.ModuleType("antenv.axon_hooks")
        _state = {"hook": None}
        mod.set_axon_ntff_profile_hook = lambda h: _state.__setitem__("hook", h)
        mod.get_axon_ntff_profile_hook = lambda: _state["hook"]
        sys.modules["antenv.axon_hooks"] = mod
        antenv.axon_hooks = mod
        from trn_agent_boot.trn_boot import _ntff_profile_via_ctypes
        mod.set_axon_ntff_profile_hook(
            _ntff_profile_via_ctypes("/opt/axon/libaxon_pjrt.so")
        )
        bu.upload_artifacts = lambda tmpdir: tmpdir
    nc = _get_nc()
    in_maps = _cache["in_maps"]
    res = run_bass_kernel_spmd(
        nc, in_maps, core_ids=list(range(N_CORES)), trace=True, trace_cores=[0]
    )
    _cache["last_trace"] = res.instructions_and_trace
    _cache["last_res"] = res
    return res.exec_time_ns
